# revision 1
# baseline (speedup 1.0000x reference)
"""AdaptiveSplineLayer on 8 Trainium2 NeuronCores (Bass/Tile).

Reference computation (per element, feature i, row m):
    sort grid[i], mc = (coeffs*sigmoid(alive)) sorted by grid order
    xn = clip((clip(x, gmin, gmax) - gmin) / range * 11, ...)
    spline = lerp of mc at floor(xn)
    out = spline @ proj_w.T + proj_b + x @ res_w.T

Kernel strategy (data-parallel over M; spline params + weights replicated):
  * Host: w = gscale*x + gbias (unclamped normalized coordinate, transposed
    to feature-major). The piecewise-linear spline with uniform knots is
    exactly  f(w) = mc0 + sum_{j=0..11} D_j * relu(w - j)  where the two end
    kinks reproduce the clamping. mc0 folds into the output bias; 1/gscale
    folds into the res weights, so the device only sees normalized inputs.
  * Device: 6 fused custom-DVE ops per feature tile evaluate all 12 kinks
    (2 kinks per 8-stage op), writing spline^T in bf16. TensorE contracts
    spline^T @ proj_w^T in bf16 and the res path in fp8e4 DoubleRow
    (centered gscale*x/64 against res_w^T/gscale*64 - 2x PE throughput),
    PSUM-accumulated, seeded by a K=1 fp16 matmul carrying the folded bias.
"""

import os
import sys

import numpy as np

for _p in ("/opt/trn_rl_repo",):
    if _p not in sys.path and os.path.isdir(_p):
        sys.path.insert(0, _p)

import ml_dtypes

BF16 = ml_dtypes.bfloat16
FP8 = ml_dtypes.float8_e4m3fn
F16 = np.float16

M, IN, OUT, K = 16384, 1024, 1024, 12
N_CORES = 8
MC = M // N_CORES  # 2048 rows per core
FT = IN // 128  # 8 feature tiles
MT = MC // 128  # 16 m tiles per core
OC = OUT // 512  # 2 output column chunks
NKINK = K  # 12 kink terms j=0..11
W8_SCALE = 64.0  # res-path fp8 input downscale (weights upscaled to match)
SPL_SCALE = 4.0  # spline fp8 upscale (proj weights downscaled to match)
FP8_SPL = True  # spline matmul path in fp8 DoubleRow (else bf16)
FP8_RES = False  # res matmul path in fp8 DoubleRow (else bf16; fp8 fails max-err)

# --------------------------------------------------------------------------
# Custom DVE ops: two relu-kinks per pass.
#   SPLINE_PAIR0   : out = s0*relu(in0-imm2) + s1*relu(in0-(imm2+1))
#   SPLINE_PAIR_ACC: out = in1 + s0*relu(in0-imm2) + s1*relu(in0-(imm2+1))
# --------------------------------------------------------------------------


def _register_spline_ops():
    from concourse.dve_ops import (
        CUSTOM_DVE_SPECS,
        OPS,
        _SUB_OPCODE_FOR_NAME,
        DveOp,
    )
    from concourse.dve_spec import (
        C0,
        C1,
        C2,
        One,
        Spec,
        Src0,
        Src1,
        _has_src1,
        lower,
        relu,
    )
    from concourse.dve_uop import DveOpSpec

    def _dve_relu(x):
        return np.maximum(
            np.nan_to_num(x, nan=0.0, posinf=np.inf, neginf=-np.inf), 0
        )

    def _ref_pair_acc(in0, in1, s0, s1, imm2):
        return (
            in1
            + s0 * _dve_relu(in0.astype(np.float32) - imm2)
            + s1 * _dve_relu(in0.astype(np.float32) - (imm2 + 1.0))
        ).astype(np.float32)

    def _ref_pair0(in0, in1, s0, s1, imm2):
        return (
            s0 * _dve_relu(in0.astype(np.float32) - imm2)
            + s1 * _dve_relu(in0.astype(np.float32) - (imm2 + 1.0))
        ).astype(np.float32)

    def _reg(name, spec):
        if name in _SUB_OPCODE_FOR_NAME:
            return next(o for o in OPS if o.name == name)
        row = max(_SUB_OPCODE_FOR_NAME.values()) + 1
        assert row < 0x20
        op = DveOp.__new__(DveOp)
        object.__setattr__(op, "name", name)
        object.__setattr__(op, "spec", spec)
        object.__setattr__(op, "subdim", False)
        object.__setattr__(op, "perf_en", {})
        s = DveOpSpec(
            name=name,
            opcode=row,
            uops=lower(spec, ver="v3"),
            rd1_en=_has_src1(spec),
        )
        object.__setattr__(op, "uops_sha", {"v3": s.sha("v3")})
        OPS.append(op)
        _SUB_OPCODE_FOR_NAME[name] = row
        CUSTOM_DVE_SPECS[name] = spec
        return op

    body_acc = (Src1 + C0 * relu(Src0 - C2)) + C1 * relu(Src0 - (C2 + One))
    body0 = C0 * relu(Src0 - C2) + C1 * relu(Src0 - (C2 + One))
    pair_acc = _reg("SPLINE_PAIR_ACC", Spec(body=body_acc, reference=_ref_pair_acc))
    pair0 = _reg("SPLINE_PAIR0", Spec(body=body0, reference=_ref_pair0))
    return pair0, pair_acc


# --------------------------------------------------------------------------
# Device graph
# --------------------------------------------------------------------------

_GRAPH_CACHE = {}


def _build_graph(m_split=(9, 7), repeat=1, w32_bufs=6, acc_bufs=6, fori=False, skip=()):
    if isinstance(m_split, (tuple, list)):
        m_split = tuple(m_split)
    key = (m_split, repeat, w32_bufs, acc_bufs, fori, tuple(skip), FP8_SPL, FP8_RES)
    if key in _GRAPH_CACHE:
        return _GRAPH_CACHE[key]

    import concourse.bacc as bacc
    import concourse.mybir as mybir
    import concourse.tile as tile

    pair0, pair_acc = _register_spline_ops()

    dt = mybir.dt
    DR = mybir.MatmulPerfMode.DoubleRow
    nc = bacc.Bacc("TRN2", target_bir_lowering=False, debug=False, num_devices=1)

    w32 = nc.dram_tensor("w32", [IN, MC], dt.float32, kind="ExternalInput")
    res_dt = dt.float8e4 if FP8_RES else dt.bfloat16
    w8 = nc.dram_tensor("w8", [IN, MC], res_dt, kind="ExternalInput")
    spl_dt = dt.float8e4 if FP8_SPL else dt.bfloat16
    wtb = nc.dram_tensor("wtb", [128, FT, OUT], spl_dt, kind="ExternalInput")
    wt8 = nc.dram_tensor("wt8", [128, FT, OUT], res_dt, kind="ExternalInput")
    bias = nc.dram_tensor("bias", [1, OUT], dt.float16, kind="ExternalInput")
    dcoef = nc.dram_tensor("dcoef", [128, FT, NKINK], dt.float32, kind="ExternalInput")
    out = nc.dram_tensor("out", [MC, OUT], dt.float32, kind="ExternalOutput")


    with tile.TileContext(nc) as tc:
        with (
            tc.tile_pool(name="const", bufs=1) as const_pool,
            tc.tile_pool(name="wtp", bufs=1) as wt_pool,
            tc.tile_pool(name="w32p", bufs=w32_bufs) as w32_pool,
            tc.tile_pool(name="persist", bufs=1) as persist_pool,
            tc.tile_pool(name="accp", bufs=acc_bufs) as acc_pool,
            tc.tile_pool(name="outp", bufs=4) as out_pool,
            tc.tile_pool(name="psum", bufs=8, space="PSUM") as psum_pool,
        ):
            # ---- small constants first (first DVE op needs dcoef) ----
            dc_t = const_pool.tile([128, FT, NKINK], dt.float32)
            nc.sync.dma_start(dc_t[:], dcoef[:])
            ones_t = const_pool.tile([1, 128], dt.float16)
            nc.vector.memset(ones_t[:], 1.0)
            bias_t = const_pool.tile([1, OUT], dt.float16)
            nc.sync.dma_start(bias_t[:], bias[:])

            wtb_t = wt_pool.tile([128, FT, OUT], spl_dt, tag="wtb")
            wt8_t = wt_pool.tile([128, FT, OUT], res_dt, tag="wt8")

            # persistent matmul inputs
            w8_t = persist_pool.tile([128, FT, MC], res_dt, tag="w8")
            spl_t = persist_pool.tile([128, FT, MC], spl_dt, tag="spl")

            from contextlib import nullcontext

            if "spline" in skip:
                nc.gpsimd.memset(spl_t[:], 0.0)
            if fori:
                # bench mode: weights are loop-invariant; load them up front
                nc.sync.dma_start(wtb_t[:], wtb[:])
                nc.sync.dma_start(wt8_t[:], wt8[:])
                for t in range(FT):
                    nc.sync.dma_start(
                        w8_t[:, t, :], w8[128 * t : 128 * (t + 1), :]
                    )
            rep_ctx = (
                tc.For_i(0, repeat, 1, staggered_reset=True)
                if fori
                else nullcontext()
            )
            blocks = (
                list(m_split)
                if isinstance(m_split, (tuple, list))
                else [MT // m_split] * m_split
            )
            assert sum(blocks) == MT
            starts = [sum(blocks[:i]) for i in range(len(blocks))]
            with rep_ctx:
              for _rep in range(1 if fori else repeat):
                for h, (b0, bn) in enumerate(zip(starts, blocks)):
                    sl = slice(b0 * 128, (b0 + bn) * 128)
                    MS = bn * 128
                    # chains in interleaved pairs: adjacent DVE ops belong to
                    # different chains, so the engine never stalls on its own
                    # predecessor's completion semaphore
                    tpairs = (
                        [(t, t + 1) for t in range(0, FT, 2)]
                        if "spline" not in skip
                        else []
                    )
                    for tp in tpairs:
                        w32s, dcss, accs = {}, {}, {}
                        for t in tp:
                            w32_tile = w32_pool.tile(
                                [128, MS], dt.float32, tag="w32"
                            )
                            w32s[t] = w32_tile
                            nc.sync.dma_start(
                                w32s[t][:], w32[128 * t : 128 * (t + 1), sl]
                            )
                            dcss[t] = [
                                dc_t[:, t, j : j + 1] for j in range(NKINK)
                            ]
                        for t in tp:
                            acc0 = acc_pool.tile(
                                [128, MS], dt.float32, tag="acc"
                            )
                            accs[t] = acc0
                            nc.vector._custom_dve(
                                pair0, out=accs[t][:], in0=w32s[t][:],
                                s0=dcss[t][0], s1=dcss[t][1], imm2=0.0,
                            )
                        for q in range(1, NKINK // 2 - 1):
                            for t in tp:
                                nxt = acc_pool.tile(
                                    [128, MS], dt.float32, tag="acc"
                                )
                                nc.vector._custom_dve(
                                    pair_acc, out=nxt[:], in0=w32s[t][:],
                                    in1=accs[t][:], s0=dcss[t][2 * q],
                                    s1=dcss[t][2 * q + 1], imm2=float(2 * q),
                                )
                                accs[t] = nxt
                        for t in tp:
                            nc.vector._custom_dve(
                                pair_acc, out=spl_t[:, t, sl], in0=w32s[t][:],
                                in1=accs[t][:], s0=dcss[t][NKINK - 2],
                                s1=dcss[t][NKINK - 1], imm2=float(NKINK - 2),
                            )
                    if h == 0 and not fori:
                        # weights + res inputs: needed only once the first
                        # matmul group runs; keep them off the critical DMA
                        # path of the first spline block
                        nc.sync.dma_start(wtb_t[:], wtb[:])
                        nc.sync.dma_start(wt8_t[:], wt8[:])
                        for t in range(FT):
                            nc.sync.dma_start(
                                w8_t[:, t, :], w8[128 * t : 128 * (t + 1), :]
                            )
                    groups = [
                        (mt, oc)
                        for mt in range(b0, b0 + bn)
                        for oc in range(OC)
                    ] if "mm" not in skip else []
                    for wv in range(0, len(groups), 8):
                        wave = groups[wv : wv + 8]
                        pss = {}
                        # phase A: bias + fp8 res path (inputs ready early)
                        for mt, oc in wave:
                            msl = slice(128 * mt, 128 * (mt + 1))
                            osl = slice(512 * oc, 512 * (oc + 1))
                            ps = psum_pool.tile([128, 512], dt.float32)
                            pss[(mt, oc)] = ps
                            nc.tensor.matmul(
                                ps[:], ones_t[:], bias_t[:, osl],
                                start=True, stop=False,
                            )
                            if FP8_RES:
                                for q in range(FT // 2):
                                    nc.tensor.matmul(
                                        ps[:],
                                        w8_t[:, 2 * q : 2 * q + 2, msl],
                                        wt8_t[:, 2 * q : 2 * q + 2, osl],
                                        start=False, stop=False,
                                        perf_mode=DR,
                                    )
                            else:
                                for t in range(FT):
                                    nc.tensor.matmul(
                                        ps[:], w8_t[:, t, msl],
                                        wt8_t[:, t, osl],
                                        start=False, stop=False,
                                    )
                        # phase B: spline path t-major, so TensorE consumes
                        # each spline chain as VectorE finishes it
                        if FP8_SPL:
                            for q in range(FT // 2):
                                for mt, oc in wave:
                                    msl = slice(128 * mt, 128 * (mt + 1))
                                    osl = slice(512 * oc, 512 * (oc + 1))
                                    nc.tensor.matmul(
                                        pss[(mt, oc)][:],
                                        spl_t[:, 2 * q : 2 * q + 2, msl],
                                        wtb_t[:, 2 * q : 2 * q + 2, osl],
                                        start=False, stop=(q == FT // 2 - 1),
                                        perf_mode=DR,
                                    )
                        else:
                            for t in range(FT):
                                for mt, oc in wave:
                                    msl = slice(128 * mt, 128 * (mt + 1))
                                    osl = slice(512 * oc, 512 * (oc + 1))
                                    nc.tensor.matmul(
                                        pss[(mt, oc)][:],
                                        spl_t[:, t, msl], wtb_t[:, t, osl],
                                        start=False, stop=(t == FT - 1),
                                    )
                        # phase C: evacuate. In the final wave VectorE is
                        # already done with splines - split the copies so the
                        # tail isn't serialized on ScalarE alone.
                        last_wave = (
                            h == len(blocks) - 1 and wv + 8 >= len(groups)
                        )
                        for gi, (mt, oc) in enumerate(wave):
                            msl = slice(128 * mt, 128 * (mt + 1))
                            osl = slice(512 * oc, 512 * (oc + 1))
                            ot = out_pool.tile([128, 512], dt.float32, tag="evac")
                            if last_wave and gi % 2 == 0:
                                nc.vector.tensor_copy(ot[:], pss[(mt, oc)][:])
                            else:
                                nc.scalar.copy(ot[:], pss[(mt, oc)][:])
                            nc.gpsimd.dma_start(out[msl, osl], ot[:])

            if "mm" in skip:
                with tc.tile_pool(name="sinkp", bufs=2) as sink_pool:
                    for mt in range(MT):
                        st = sink_pool.tile([128, OUT], dt.float32, tag="sink")
                        nc.scalar.copy(st[:], spl_t[:, mt % FT, 0:OUT])
                        nc.sync.dma_start(out[128 * mt : 128 * (mt + 1), :], st[:])

    nc.compile()
    _GRAPH_CACHE[key] = nc
    return nc


# --------------------------------------------------------------------------
# Host-side parameter preparation
# --------------------------------------------------------------------------


def _prep(x, grid, coeffs, knot_alive, proj_w, proj_b, res_w):
    g64 = grid.astype(np.float64)
    order = np.argsort(g64, axis=1, kind="stable")
    sg = np.take_along_axis(grid.astype(np.float32), order, axis=1)
    # masked coeffs, sorted by grid order (sigmoid in f32 like the reference)
    mcu = coeffs.astype(np.float32) * (
        1.0 / (1.0 + np.exp(-knot_alive.astype(np.float32)))
    )
    mc = np.take_along_axis(mcu, order, axis=1).astype(np.float64)  # (IN, K)

    gmin = sg[:, 0].astype(np.float64)
    gmax = sg[:, -1].astype(np.float64)
    rng = np.maximum(gmax - gmin, 1e-6)
    gscale = (K - 1) / rng  # (IN,)
    gbias = -gmin * gscale

    # kink decomposition: f(w) = mc0 + sum_{j=0..11} D_j relu(w - j)
    s = mc[:, 1:] - mc[:, :-1]  # slopes, (IN, 11)
    D = np.empty((IN, NKINK), dtype=np.float64)
    D[:, 0] = s[:, 0]
    D[:, 1:11] = s[:, 1:] - s[:, :-1]
    D[:, 11] = -s[:, -1]
    if FP8_SPL:
        D *= SPL_SCALE  # device computes SPL_SCALE*spline; wtb is descaled

    # normalized coordinates, feature-major
    wc = (x.astype(np.float64) * gscale[None, :]).T  # centered, (IN, M)
    w32 = np.ascontiguousarray(wc + gbias[:, None], dtype=np.float32)
    if FP8_RES:
        w8 = (wc / W8_SCALE).astype(FP8)
    else:
        w8 = wc.astype(BF16)

    pwT = proj_w.astype(np.float64).T  # (IN, OUT)
    rwT = res_w.astype(np.float64).T  # (IN, OUT)
    def _tile_rows(a, dtype):
        return np.ascontiguousarray(
            a.reshape(FT, 128, OUT).transpose(1, 0, 2), dtype=dtype
        )

    if FP8_SPL:
        wtb = _tile_rows(pwT / SPL_SCALE, FP8)
    else:
        wtb = _tile_rows(pwT, BF16)
    if FP8_RES:
        wt8 = _tile_rows(rwT / gscale[:, None] * W8_SCALE, FP8)
    else:
        wt8 = _tile_rows(rwT / gscale[:, None], BF16)

    bfold = proj_b.astype(np.float64) + mc[:, 0] @ pwT
    bias = np.ascontiguousarray(bfold[None, :], dtype=F16)

    dcoef = np.ascontiguousarray(
        D.reshape(FT, 128, NKINK).transpose(1, 0, 2), dtype=np.float32
    )
    return w32, w8, wtb, wt8, bias, dcoef


def _make_in_maps(inputs):
    w32, w8, wtb, wt8, bias, dcoef = _prep(**inputs)
    in_maps = []
    for c in range(N_CORES):
        sl = slice(c * MC, (c + 1) * MC)
        in_maps.append(
            {
                "w32": np.ascontiguousarray(w32[:, sl]),
                "w8": np.ascontiguousarray(w8[:, sl]),
                "wtb": wtb,
                "wt8": wt8,
                "bias": bias,
                "dcoef": dcoef,
            }
        )
    return in_maps


def kernel(**inputs):
    from concourse.bass_utils import run_bass_kernel_spmd

    inputs = {k: np.asarray(v) for k, v in inputs.items()}
    nc = _build_graph()
    in_maps = _make_in_maps(inputs)
    res = run_bass_kernel_spmd(nc, in_maps, core_ids=list(range(N_CORES)))
    return np.concatenate(
        [res.results[c]["out"] for c in range(N_CORES)], axis=0
    )


if __name__ == "__main__":
    rng = np.random.default_rng(0)
    fake = {
        "x": rng.standard_normal((M, IN), dtype=np.float32),
        "grid": rng.standard_normal((IN, K), dtype=np.float32),
        "coeffs": rng.standard_normal((IN, K), dtype=np.float32) * 0.1,
        "knot_alive": rng.standard_normal((IN, K), dtype=np.float32) + 3,
        "proj_w": rng.standard_normal((OUT, IN), dtype=np.float32) / 32,
        "proj_b": rng.standard_normal((OUT,), dtype=np.float32) * 0.01,
        "res_w": rng.standard_normal((OUT, IN), dtype=np.float32) / 32,
    }
    y = kernel(**fake)
    print("kernel output", y.shape, y.dtype)



# revision 2
# speedup vs baseline: 1.0241x; 1.0241x over previous
"""AdaptiveSplineLayer on 8 Trainium2 NeuronCores (Bass/Tile).

Reference computation (per element, feature i, row m):
    sort grid[i], mc = (coeffs*sigmoid(alive)) sorted by grid order
    xn = clip((clip(x, gmin, gmax) - gmin) / range * 11, ...)
    spline = lerp of mc at floor(xn)
    out = spline @ proj_w.T + proj_b + x @ res_w.T

Kernel strategy (data-parallel over M; spline params + weights replicated):
  * Host: w = gscale*x + gbias (unclamped normalized coordinate, fp16,
    feature-major). The piecewise-linear spline with uniform knots is exactly
    f(w) = mc0 + sum_{j=0..11} D_j * relu(w - j); the two end kinks reproduce
    the clamping. mc0 folds into the output bias.
  * The SAME w tensor feeds the residual path: x = (w - gbias)/gscale, so
    x @ res_w.T = w @ (res_w/gscale).T + gmin @ res_w.T (constant -> bias).
    One 4MB fp16 input replaces the baseline's 8MB f32 + 4MB bf16 pair.
  * Device: 6 custom-DVE pair ops per feature tile evaluate all 12 kinks
    (2 kinks per 8-stage op), writing spline^T in fp8e4. TensorE contracts
    spline^T @ (proj_w/4)^T in fp8 DoubleRow and w^T @ (res_w/gscale)^T in
    fp16, PSUM-accumulated, seeded by a K=1 fp16 matmul carrying the folded
    bias. Output is written fp16 (4MB) and upcast to f32 on host.
"""

import os
import sys

import numpy as np

for _p in ("/opt/trn_rl_repo",):
    if _p not in sys.path and os.path.isdir(_p):
        sys.path.insert(0, _p)

import ml_dtypes

BF16 = ml_dtypes.bfloat16
FP8 = ml_dtypes.float8_e4m3fn
F16 = np.float16

M, IN, OUT, K = 16384, 1024, 1024, 12
N_CORES = 8
MC = M // N_CORES  # 2048 rows per core
FT = IN // 128  # 8 feature tiles
MT = MC // 128  # 16 m tiles per core
OC = OUT // 512  # 2 output column chunks
NKINK = K  # 12 kink terms j=0..11
SPL_SCALE = 4.0  # spline fp8 upscale (proj weights descaled to match)

# --------------------------------------------------------------------------
# Custom DVE ops: two relu-kinks per pass.
#   SPLINE_PAIR0   : out = s0*relu(in0-imm2) + s1*relu(in0-(imm2+1))
#   SPLINE_PAIR_ACC: out = in1 + s0*relu(in0-imm2) + s1*relu(in0-(imm2+1))
# --------------------------------------------------------------------------


def _register_spline_ops():
    from concourse.dve_ops import (
        CUSTOM_DVE_SPECS,
        OPS,
        _SUB_OPCODE_FOR_NAME,
        DveOp,
    )
    from concourse.dve_spec import (
        C0,
        C1,
        C2,
        One,
        Spec,
        Src0,
        Src1,
        _has_src1,
        lower,
        relu,
    )
    from concourse.dve_uop import DveOpSpec

    def _dve_relu(x):
        return np.maximum(
            np.nan_to_num(x, nan=0.0, posinf=np.inf, neginf=-np.inf), 0
        )

    def _ref_pair_acc(in0, in1, s0, s1, imm2):
        return (
            in1
            + s0 * _dve_relu(in0.astype(np.float32) - imm2)
            + s1 * _dve_relu(in0.astype(np.float32) - (imm2 + 1.0))
        ).astype(np.float32)

    def _ref_pair0(in0, in1, s0, s1, imm2):
        return (
            s0 * _dve_relu(in0.astype(np.float32) - imm2)
            + s1 * _dve_relu(in0.astype(np.float32) - (imm2 + 1.0))
        ).astype(np.float32)

    def _reg(name, spec):
        if name in _SUB_OPCODE_FOR_NAME:
            return next(o for o in OPS if o.name == name)
        row = max(_SUB_OPCODE_FOR_NAME.values()) + 1
        assert row < 0x20
        op = DveOp.__new__(DveOp)
        object.__setattr__(op, "name", name)
        object.__setattr__(op, "spec", spec)
        object.__setattr__(op, "subdim", False)
        object.__setattr__(op, "perf_en", {})
        s = DveOpSpec(
            name=name,
            opcode=row,
            uops=lower(spec, ver="v3"),
            rd1_en=_has_src1(spec),
        )
        object.__setattr__(op, "uops_sha", {"v3": s.sha("v3")})
        OPS.append(op)
        _SUB_OPCODE_FOR_NAME[name] = row
        CUSTOM_DVE_SPECS[name] = spec
        return op

    body_acc = (Src1 + C0 * relu(Src0 - C2)) + C1 * relu(Src0 - (C2 + One))
    body0 = C0 * relu(Src0 - C2) + C1 * relu(Src0 - (C2 + One))
    pair_acc = _reg("SPLINE_PAIR_ACC", Spec(body=body_acc, reference=_ref_pair_acc))
    pair0 = _reg("SPLINE_PAIR0", Spec(body=body0, reference=_ref_pair0))
    return pair0, pair_acc


# --------------------------------------------------------------------------
# Device graph
# --------------------------------------------------------------------------

_GRAPH_CACHE = {}


def _build_graph(m_split=(9, 7), repeat=1, acc_bufs=6, fori=False, skip=()):
    if isinstance(m_split, (tuple, list)):
        m_split = tuple(m_split)
    key = (m_split, repeat, acc_bufs, fori, tuple(skip))
    if key in _GRAPH_CACHE:
        return _GRAPH_CACHE[key]

    import concourse.bacc as bacc
    import concourse.mybir as mybir
    import concourse.tile as tile

    pair0, pair_acc = _register_spline_ops()

    dt = mybir.dt
    DR = mybir.MatmulPerfMode.DoubleRow
    nc = bacc.Bacc("TRN2", target_bir_lowering=False, debug=False, num_devices=1)

    w16 = nc.dram_tensor("w16", [IN, MC], dt.float16, kind="ExternalInput")
    wtb = nc.dram_tensor("wtb", [128, FT, OUT], dt.float8e4, kind="ExternalInput")
    wt8 = nc.dram_tensor("wt8", [128, FT, OUT], dt.float16, kind="ExternalInput")
    bias = nc.dram_tensor("bias", [1, OUT], dt.float16, kind="ExternalInput")
    dcoef = nc.dram_tensor("dcoef", [128, FT, NKINK], dt.float32, kind="ExternalInput")
    out = nc.dram_tensor("out", [MC, OUT], dt.float16, kind="ExternalOutput")

    with tile.TileContext(nc) as tc:
        with (
            tc.tile_pool(name="const", bufs=1) as const_pool,
            tc.tile_pool(name="wtp", bufs=1) as wt_pool,
            tc.tile_pool(name="persist", bufs=1) as persist_pool,
            tc.tile_pool(name="accp", bufs=acc_bufs) as acc_pool,
            tc.tile_pool(name="outp", bufs=4) as out_pool,
            tc.tile_pool(name="psum", bufs=8, space="PSUM") as psum_pool,
        ):
            # ---- small constants first (first DVE op needs dcoef) ----
            dc_t = const_pool.tile([128, FT, NKINK], dt.float32)
            nc.sync.dma_start(dc_t[:], dcoef[:])
            ones_t = const_pool.tile([1, 128], dt.float16)
            nc.vector.memset(ones_t[:], 1.0)
            bias_t = const_pool.tile([1, OUT], dt.float16)
            nc.sync.dma_start(bias_t[:], bias[:])

            wtb_t = wt_pool.tile([128, FT, OUT], dt.float8e4, tag="wtb")
            wt8_t = wt_pool.tile([128, FT, OUT], dt.float16, tag="wt8")

            # persistent matmul inputs: w16 doubles as DVE input + res operand
            w16_t = persist_pool.tile([128, FT, MC], dt.float16, tag="w16")
            spl_t = persist_pool.tile([128, FT, MC], dt.float8e4, tag="spl")

            from contextlib import nullcontext

            if "spline" in skip:
                nc.gpsimd.memset(spl_t[:], 0.0)
            if fori:
                # bench mode: weights are loop-invariant; load them up front
                nc.sync.dma_start(wtb_t[:], wtb[:])
                nc.sync.dma_start(wt8_t[:], wt8[:])
            rep_ctx = (
                tc.For_i(0, repeat, 1, staggered_reset=True)
                if fori
                else nullcontext()
            )
            blocks = (
                list(m_split)
                if isinstance(m_split, (tuple, list))
                else [MT // m_split] * m_split
            )
            assert sum(blocks) == MT
            starts = [sum(blocks[:i]) for i in range(len(blocks))]
            with rep_ctx:
              for _rep in range(1 if fori else repeat):
                for h, (b0, bn) in enumerate(zip(starts, blocks)):
                    sl = slice(b0 * 128, (b0 + bn) * 128)
                    MS = bn * 128
                    # chains in interleaved pairs: adjacent DVE ops belong to
                    # different chains, so the engine never stalls on its own
                    # predecessor's completion semaphore
                    tpairs = (
                        [(t, t + 1) for t in range(0, FT, 2)]
                        if "spline" not in skip
                        else []
                    )
                    for tp in tpairs:
                        dcss, accs = {}, {}
                        for t in tp:
                            nc.sync.dma_start(
                                w16_t[:, t, sl], w16[128 * t : 128 * (t + 1), sl]
                            )
                            dcss[t] = [
                                dc_t[:, t, j : j + 1] for j in range(NKINK)
                            ]
                        for t in tp:
                            acc0 = acc_pool.tile(
                                [128, MS], dt.float32, tag="acc"
                            )
                            accs[t] = acc0
                            nc.vector._custom_dve(
                                pair0, out=accs[t][:], in0=w16_t[:, t, sl],
                                s0=dcss[t][0], s1=dcss[t][1], imm2=0.0,
                            )
                        for q in range(1, NKINK // 2 - 1):
                            for t in tp:
                                nxt = acc_pool.tile(
                                    [128, MS], dt.float32, tag="acc"
                                )
                                nc.vector._custom_dve(
                                    pair_acc, out=nxt[:], in0=w16_t[:, t, sl],
                                    in1=accs[t][:], s0=dcss[t][2 * q],
                                    s1=dcss[t][2 * q + 1], imm2=float(2 * q),
                                )
                                accs[t] = nxt
                        for t in tp:
                            nc.vector._custom_dve(
                                pair_acc, out=spl_t[:, t, sl], in0=w16_t[:, t, sl],
                                in1=accs[t][:], s0=dcss[t][NKINK - 2],
                                s1=dcss[t][NKINK - 1], imm2=float(NKINK - 2),
                            )
                    if h == 0 and not fori:
                        # weights: needed only once the first matmul group
                        # runs; keep them off the critical DMA path of the
                        # first spline block
                        nc.sync.dma_start(wtb_t[:], wtb[:])
                        nc.sync.dma_start(wt8_t[:], wt8[:])
                    # matmul waves: 8 PSUM banks; groups ordered so both oc
                    # chunks of an mt land in the same wave (merged evac)
                    groups = [
                        (mt, oc)
                        for mt in range(b0, b0 + bn)
                        for oc in range(OC)
                    ] if "mm" not in skip else []
                    for wv in range(0, len(groups), 8):
                        wave = groups[wv : wv + 8]
                        pss = {}
                        # phase A: bias seed + fp16 res path (input is ready)
                        for mt, oc in wave:
                            msl = slice(128 * mt, 128 * (mt + 1))
                            osl = slice(512 * oc, 512 * (oc + 1))
                            ps = psum_pool.tile([128, 512], dt.float32)
                            pss[(mt, oc)] = ps
                            nc.tensor.matmul(
                                ps[:], ones_t[:], bias_t[:, osl],
                                start=True, stop=False,
                            )
                            for t in range(FT):
                                nc.tensor.matmul(
                                    ps[:], w16_t[:, t, msl],
                                    wt8_t[:, t, osl],
                                    start=False, stop=False,
                                )
                        # phase B: spline path t-major, so TensorE consumes
                        # each spline chain as VectorE finishes it
                        for q in range(FT // 2):
                            for mt, oc in wave:
                                msl = slice(128 * mt, 128 * (mt + 1))
                                osl = slice(512 * oc, 512 * (oc + 1))
                                nc.tensor.matmul(
                                    pss[(mt, oc)][:],
                                    spl_t[:, 2 * q : 2 * q + 2, msl],
                                    wtb_t[:, 2 * q : 2 * q + 2, osl],
                                    start=False, stop=(q == FT // 2 - 1),
                                    perf_mode=DR,
                                )
                        # phase C: evacuate both oc chunks of an mt into one
                        # [128, OUT] fp16 staging tile -> 2KB DMA lines.
                        last_wave = (
                            h == len(blocks) - 1 and wv + 8 >= len(groups)
                        )
                        mts = sorted({mt for mt, _ in wave})
                        for gi, mt in enumerate(mts):
                            msl = slice(128 * mt, 128 * (mt + 1))
                            ot = out_pool.tile([128, OUT], dt.float16, tag="evac")
                            for oc in range(OC):
                                osl = slice(512 * oc, 512 * (oc + 1))
                                if last_wave and (gi + oc) % 2 == 0:
                                    nc.vector.tensor_copy(
                                        ot[:, osl], pss[(mt, oc)][:]
                                    )
                                else:
                                    nc.scalar.copy(ot[:, osl], pss[(mt, oc)][:])
                            nc.gpsimd.dma_start(out[msl, :], ot[:])

            if "mm" in skip:
                with tc.tile_pool(name="sinkp", bufs=2) as sink_pool:
                    for mt in range(MT):
                        st = sink_pool.tile([128, OUT], dt.float16, tag="sink")
                        nc.scalar.copy(st[:], spl_t[:, mt % FT, 0:OUT])
                        nc.sync.dma_start(out[128 * mt : 128 * (mt + 1), :], st[:])

    nc.compile()
    _GRAPH_CACHE[key] = nc
    return nc


# --------------------------------------------------------------------------
# Host-side parameter preparation
# --------------------------------------------------------------------------


def _prep(x, grid, coeffs, knot_alive, proj_w, proj_b, res_w):
    g64 = grid.astype(np.float64)
    order = np.argsort(g64, axis=1, kind="stable")
    sg = np.take_along_axis(grid.astype(np.float32), order, axis=1)
    # masked coeffs, sorted by grid order (sigmoid in f32 like the reference)
    mcu = coeffs.astype(np.float32) * (
        1.0 / (1.0 + np.exp(-knot_alive.astype(np.float32)))
    )
    mc = np.take_along_axis(mcu, order, axis=1).astype(np.float64)  # (IN, K)

    gmin = sg[:, 0].astype(np.float64)
    gmax = sg[:, -1].astype(np.float64)
    rng = np.maximum(gmax - gmin, 1e-6)
    gscale = (K - 1) / rng  # (IN,)
    gbias = -gmin * gscale

    # kink decomposition: f(w) = mc0 + sum_{j=0..11} D_j relu(w - j)
    s = mc[:, 1:] - mc[:, :-1]  # slopes, (IN, 11)
    D = np.empty((IN, NKINK), dtype=np.float64)
    D[:, 0] = s[:, 0]
    D[:, 1:11] = s[:, 1:] - s[:, :-1]
    D[:, 11] = -s[:, -1]
    D *= SPL_SCALE  # device computes SPL_SCALE*spline; wtb is descaled

    # normalized coordinate, feature-major; consumed by the spline chain AND
    # (rescaled weights) by the residual matmul
    w = (x.astype(np.float64) * gscale[None, :] + gbias[None, :]).T  # (IN, M)
    w16 = np.ascontiguousarray(w, dtype=F16)

    pwT = proj_w.astype(np.float64).T  # (IN, OUT)
    rwT = res_w.astype(np.float64).T  # (IN, OUT)

    def _tile_rows(a, dtype):
        return np.ascontiguousarray(
            a.reshape(FT, 128, OUT).transpose(1, 0, 2), dtype=dtype
        )

    wtb = _tile_rows(pwT / SPL_SCALE, FP8)
    wt8 = _tile_rows(rwT / gscale[:, None], F16)

    # bias fold: proj_b + mc0 @ pwT + gmin @ rwT (res path shift)
    bfold = proj_b.astype(np.float64) + mc[:, 0] @ pwT + gmin @ rwT
    bias = np.ascontiguousarray(bfold[None, :], dtype=F16)

    dcoef = np.ascontiguousarray(
        D.reshape(FT, 128, NKINK).transpose(1, 0, 2), dtype=np.float32
    )
    return w16, wtb, wt8, bias, dcoef


def _make_in_maps(inputs):
    w16, wtb, wt8, bias, dcoef = _prep(**inputs)
    in_maps = []
    for c in range(N_CORES):
        sl = slice(c * MC, (c + 1) * MC)
        in_maps.append(
            {
                "w16": np.ascontiguousarray(w16[:, sl]),
                "wtb": wtb,
                "wt8": wt8,
                "bias": bias,
                "dcoef": dcoef,
            }
        )
    return in_maps


def kernel(**inputs):
    from concourse.bass_utils import run_bass_kernel_spmd

    inputs = {k: np.asarray(v) for k, v in inputs.items()}
    nc = _build_graph()
    in_maps = _make_in_maps(inputs)
    res = run_bass_kernel_spmd(nc, in_maps, core_ids=list(range(N_CORES)))
    return np.concatenate(
        [res.results[c]["out"].astype(np.float32) for c in range(N_CORES)],
        axis=0,
    )


if __name__ == "__main__":
    rng = np.random.default_rng(0)
    fake = {
        "x": rng.standard_normal((M, IN), dtype=np.float32),
        "grid": rng.standard_normal((IN, K), dtype=np.float32),
        "coeffs": rng.standard_normal((IN, K), dtype=np.float32) * 0.1,
        "knot_alive": rng.standard_normal((IN, K), dtype=np.float32) + 3,
        "proj_w": rng.standard_normal((OUT, IN), dtype=np.float32) / 32,
        "proj_b": rng.standard_normal((OUT,), dtype=np.float32) * 0.01,
        "res_w": rng.standard_normal((IN, OUT), dtype=np.float32).T / 32,
    }
    y = kernel(**fake)
    print("kernel output", y.shape, y.dtype)


# revision 12
# speedup vs baseline: 1.0586x; 1.0337x over previous
"""AdaptiveSplineLayer on 8 Trainium2 NeuronCores (Bass/Tile).

Reference computation (per element, feature i, row m):
    sort grid[i], mc = (coeffs*sigmoid(alive)) sorted by grid order
    xn = clip((clip(x, gmin, gmax) - gmin) / range * 11, ...)
    spline = lerp of mc at floor(xn)
    out = spline @ proj_w.T + proj_b + x @ res_w.T

Kernel strategy (data-parallel over M; spline params + weights replicated):
  * Host: w = gscale*x + gbias (unclamped normalized coordinate, fp16,
    feature-major). The piecewise-linear spline with uniform knots is exactly
    f(w) = mc0 + sum_{j=0..11} D_j * relu(w - j); the two end kinks reproduce
    the clamping. mc0 folds into the output bias.
  * The SAME w tensor feeds the residual path: x = (w - gbias)/gscale, so
    x @ res_w.T = w @ (res_w/gscale).T + gmin @ res_w.T (constant -> bias).
    One 4MB fp16 input replaces the baseline's 8MB f32 + 4MB bf16 pair.
  * Device: 6 custom-DVE pair ops per feature tile evaluate all 12 kinks
    (2 kinks per 8-stage op), writing spline^T in fp8e4. TensorE contracts
    spline^T @ (proj_w/4)^T in fp8 DoubleRow and w^T @ (res_w/gscale)^T in
    fp16, PSUM-accumulated, seeded by a K=1 fp16 matmul carrying the folded
    bias. Output is written fp16 (4MB) and upcast to f32 on host.
"""

import os
import sys

import numpy as np

for _p in ("/opt/trn_rl_repo",):
    if _p not in sys.path and os.path.isdir(_p):
        sys.path.insert(0, _p)

import ml_dtypes

BF16 = ml_dtypes.bfloat16
FP8 = ml_dtypes.float8_e4m3fn
F16 = np.float16

M, IN, OUT, K = 16384, 1024, 1024, 12
N_CORES = 8
MC = M // N_CORES  # 2048 rows per core
FT = IN // 128  # 8 feature tiles
MT = MC // 128  # 16 m tiles per core
OC = OUT // 512  # 2 output column chunks
NKINK = K  # 12 kink terms j=0..11
SPL_SCALE = 4.0  # spline fp8 upscale (proj weights descaled to match)

# --------------------------------------------------------------------------
# Custom DVE ops: two relu-kinks per pass.
#   SPLINE_PAIR0   : out = s0*relu(in0-imm2) + s1*relu(in0-(imm2+1))
#   SPLINE_PAIR_ACC: out = in1 + s0*relu(in0-imm2) + s1*relu(in0-(imm2+1))
# --------------------------------------------------------------------------


def _register_spline_ops():
    from concourse.dve_ops import (
        CUSTOM_DVE_SPECS,
        OPS,
        _SUB_OPCODE_FOR_NAME,
        DveOp,
    )
    from concourse.dve_spec import (
        C0,
        C1,
        C2,
        One,
        Spec,
        Src0,
        Src1,
        _has_src1,
        lower,
        relu,
    )
    from concourse.dve_uop import DveOpSpec

    def _dve_relu(x):
        return np.maximum(
            np.nan_to_num(x, nan=0.0, posinf=np.inf, neginf=-np.inf), 0
        )

    def _ref_pair_acc(in0, in1, s0, s1, imm2):
        return (
            in1
            + s0 * _dve_relu(in0.astype(np.float32) - imm2)
            + s1 * _dve_relu(in0.astype(np.float32) - (imm2 + 1.0))
        ).astype(np.float32)

    def _ref_pair0(in0, in1, s0, s1, imm2):
        return (
            s0 * _dve_relu(in0.astype(np.float32) - imm2)
            + s1 * _dve_relu(in0.astype(np.float32) - (imm2 + 1.0))
        ).astype(np.float32)

    def _reg(name, spec):
        if name in _SUB_OPCODE_FOR_NAME:
            return next(o for o in OPS if o.name == name)
        row = max(_SUB_OPCODE_FOR_NAME.values()) + 1
        assert row < 0x20
        op = DveOp.__new__(DveOp)
        object.__setattr__(op, "name", name)
        object.__setattr__(op, "spec", spec)
        object.__setattr__(op, "subdim", False)
        object.__setattr__(op, "perf_en", {})
        s = DveOpSpec(
            name=name,
            opcode=row,
            uops=lower(spec, ver="v3"),
            rd1_en=_has_src1(spec),
        )
        object.__setattr__(op, "uops_sha", {"v3": s.sha("v3")})
        OPS.append(op)
        _SUB_OPCODE_FOR_NAME[name] = row
        CUSTOM_DVE_SPECS[name] = spec
        return op

    body_acc = (Src1 + C0 * relu(Src0 - C2)) + C1 * relu(Src0 - (C2 + One))
    body0 = C0 * relu(Src0 - C2) + C1 * relu(Src0 - (C2 + One))
    pair_acc = _reg("SPLINE_PAIR_ACC", Spec(body=body_acc, reference=_ref_pair_acc))
    pair0 = _reg("SPLINE_PAIR0", Spec(body=body0, reference=_ref_pair0))
    return pair0, pair_acc


# --------------------------------------------------------------------------
# Device graph
# --------------------------------------------------------------------------

_GRAPH_CACHE = {}


def _build_graph(m_split=(8, 8), repeat=1, acc_bufs=6, fori=False, skip=(),
                 wave_n=8, act_kinks=True):
    if isinstance(m_split, (tuple, list)):
        m_split = tuple(m_split)
    key = (m_split, repeat, acc_bufs, fori, tuple(skip), wave_n, act_kinks)
    if key in _GRAPH_CACHE:
        return _GRAPH_CACHE[key]

    import concourse.bacc as bacc
    import concourse.mybir as mybir
    import concourse.tile as tile

    pair0, pair_acc = _register_spline_ops()

    dt = mybir.dt
    DR = mybir.MatmulPerfMode.DoubleRow
    nc = bacc.Bacc("TRN2", target_bir_lowering=False, debug=False, num_devices=1)

    w16 = nc.dram_tensor("w16", [IN, MC], dt.float16, kind="ExternalInput")
    wtb = nc.dram_tensor("wtb", [128, FT, OUT], dt.float8e4, kind="ExternalInput")
    wt8 = nc.dram_tensor("wt8", [128, FT, OUT], dt.float16, kind="ExternalInput")
    bias = nc.dram_tensor("bias", [128, FT], dt.float32, kind="ExternalInput")
    dcoef = nc.dram_tensor("dcoef", [128, FT, NKINK], dt.float32, kind="ExternalInput")
    out = nc.dram_tensor("out", [OUT, MC], dt.float16, kind="ExternalOutput")

    with tile.TileContext(nc) as tc:
        with (
            tc.tile_pool(name="const", bufs=1) as const_pool,
            tc.tile_pool(name="wtp", bufs=1) as wt_pool,
            tc.tile_pool(name="persist", bufs=1) as persist_pool,
            tc.tile_pool(name="accp", bufs=acc_bufs) as acc_pool,
            tc.tile_pool(name="outp", bufs=4) as out_pool,
            tc.tile_pool(name="psum", bufs=8, space="PSUM") as psum_pool,
        ):
            # ---- small constants first (first DVE op needs dcoef) ----
            dc_t = const_pool.tile([128, FT, NKINK], dt.float32)
            nc.sync.dma_start(dc_t[:], dcoef[:])
            bias_t = const_pool.tile([128, FT], dt.float32)
            nc.sync.dma_start(bias_t[:], bias[:])
            mj_t = const_pool.tile([128, 2], dt.float32)
            nc.vector.memset(mj_t[:, 0:1], -4.0)
            nc.vector.memset(mj_t[:, 1:2], -5.0)

            wtb_t = wt_pool.tile([128, FT, OUT], dt.float8e4, tag="wtb")
            wt8_t = wt_pool.tile([128, FT, OUT], dt.float16, tag="wt8")

            # persistent matmul inputs: w16 doubles as DVE input + res operand
            w16_t = persist_pool.tile([128, FT, MC], dt.float16, tag="w16")
            spl_t = persist_pool.tile([128, FT, MC], dt.float8e4, tag="spl")

            from contextlib import nullcontext

            if "spline" in skip:
                nc.gpsimd.memset(spl_t[:], 0.0)
            if fori:
                # bench mode: weights are loop-invariant; load them up front
                nc.sync.dma_start(wtb_t[:], wtb[:])
                nc.sync.dma_start(wt8_t[:], wt8[:])
            rep_ctx = (
                tc.For_i(0, repeat, 1, staggered_reset=True)
                if fori
                else nullcontext()
            )
            blocks = (
                list(m_split)
                if isinstance(m_split, (tuple, list))
                else [MT // m_split] * m_split
            )
            assert sum(blocks) == MT
            starts = [sum(blocks[:i]) for i in range(len(blocks))]
            with rep_ctx:
              for _rep in range(1 if fori else repeat):
                for h, (b0, bn) in enumerate(zip(starts, blocks)):
                    sl = slice(b0 * 128, (b0 + bn) * 128)
                    MS = bn * 128
                    # chains in interleaved pairs: adjacent DVE ops belong to
                    # different chains, so the engine never stalls on its own
                    # predecessor's completion semaphore
                    tpairs = (
                        [(t, t + 1) for t in range(0, FT, 2)]
                        if "spline" not in skip
                        else []
                    )
                    if "spline" in skip:
                        # still stream w16 (res matmul input + DMA parity)
                        for t in range(FT):
                            nc.sync.dma_start(
                                w16_t[:, t, sl], w16[128 * t : 128 * (t + 1), sl]
                            )
                    for tp in tpairs:
                        dcss, accs = {}, {}
                        for t in tp:
                            nc.sync.dma_start(
                                w16_t[:, t, sl], w16[128 * t : 128 * (t + 1), sl]
                            )
                            dcss[t] = [
                                dc_t[:, t, j : j + 1] for j in range(NKINK)
                            ]
                        for t in tp:
                            acc0 = acc_pool.tile(
                                [128, MS], dt.float32, tag="acc"
                            )
                            accs[t] = acc0
                            nc.vector._custom_dve(
                                pair0, out=accs[t][:], in0=w16_t[:, t, sl],
                                s0=dcss[t][0], s1=dcss[t][1], imm2=0.0,
                            )
                        for q in range(1, NKINK // 2 - 1):
                            for t in tp:
                                nxt = acc_pool.tile(
                                    [128, MS], dt.float32, tag="acc"
                                )
                                nc.vector._custom_dve(
                                    pair_acc, out=nxt[:], in0=w16_t[:, t, sl],
                                    in1=accs[t][:], s0=dcss[t][2 * q],
                                    s1=dcss[t][2 * q + 1], imm2=float(2 * q),
                                )
                                accs[t] = nxt
                        for t in tp:
                            nc.vector._custom_dve(
                                pair_acc, out=spl_t[:, t, sl], in0=w16_t[:, t, sl],
                                in1=accs[t][:], s0=dcss[t][NKINK - 2],
                                s1=dcss[t][NKINK - 1], imm2=float(NKINK - 2),
                            )
                    if h == 0 and not fori:
                        # weights: needed only once the first matmul group
                        # runs; keep them off the critical DMA path of the
                        # first spline block
                        nc.sync.dma_start(wtb_t[:], wtb[:])
                        nc.sync.dma_start(wt8_t[:], wt8[:])
                    # matmuls, weights-stationary: out^T[o, m] so each
                    # Ldweights (wt8/wtb column block) is reused across the
                    # block's m chunks; bias rides the ACT evacuation as a
                    # per-partition add (no bias matmul).
                    assert bn % 4 == 0, "blocks must align to 512-row chunks"
                    mchunks = [
                        slice(b0 * 128 + 512 * c, b0 * 128 + 512 * (c + 1))
                        for c in range(bn // 4)
                    ] if "mm" not in skip else []
                    ots = {}
                    for ob in range(FT):  # 8 output-column blocks of 128
                        opart = slice(128 * ob, 128 * (ob + 1))
                        if "mm" in skip:
                            break
                        ot = out_pool.tile(
                            [128, bn * 128], dt.float16, tag="evac"
                        )
                        ots[ob] = ot
                        pss = {}
                        for ci, msl in enumerate(mchunks):
                            ps = psum_pool.tile([128, 512], dt.float32)
                            pss[ci] = ps
                            # res path: one wt8 column-block load per t,
                            # streamed over this m chunk
                            for t in range(FT):
                                nc.tensor.matmul(
                                    ps[:], wt8_t[:, t, opart],
                                    w16_t[:, t, msl],
                                    start=(t == 0), stop=False,
                                )
                            # spline path fp8 DoubleRow
                            for q in range(FT // 2):
                                nc.tensor.matmul(
                                    ps[:],
                                    wtb_t[:, 2 * q : 2 * q + 2, opart],
                                    spl_t[:, 2 * q : 2 * q + 2, msl],
                                    start=False, stop=(q == FT // 2 - 1),
                                    perf_mode=DR,
                                )
                        # evacuate: ACT Identity adds per-partition bias and
                        # casts fp32->fp16 into the block's staging columns
                        for ci, msl in enumerate(mchunks):
                            nc.scalar.activation(
                                ots[ob][:, 512 * ci : 512 * (ci + 1)],
                                pss[ci][:],
                                mybir.ActivationFunctionType.Identity,
                                bias=bias_t[:, ob : ob + 1],
                            )
                        nc.gpsimd.dma_start(
                            out[opart, b0 * 128 : (b0 + bn) * 128], ots[ob][:]
                        )

            if "mm" in skip:
                with tc.tile_pool(name="sinkp", bufs=2) as sink_pool:
                    for ob in range(FT):
                        st = sink_pool.tile([128, MC], dt.float16, tag="sink")
                        nc.scalar.copy(st[:], spl_t[:, ob, :])
                        nc.sync.dma_start(out[128 * ob : 128 * (ob + 1), :], st[:])

    nc.compile()
    _GRAPH_CACHE[key] = nc
    return nc


# --------------------------------------------------------------------------
# Host-side parameter preparation
# --------------------------------------------------------------------------


def _prep(x, grid, coeffs, knot_alive, proj_w, proj_b, res_w):
    g64 = grid.astype(np.float64)
    order = np.argsort(g64, axis=1, kind="stable")
    sg = np.take_along_axis(grid.astype(np.float32), order, axis=1)
    # masked coeffs, sorted by grid order (sigmoid in f32 like the reference)
    mcu = coeffs.astype(np.float32) * (
        1.0 / (1.0 + np.exp(-knot_alive.astype(np.float32)))
    )
    mc = np.take_along_axis(mcu, order, axis=1).astype(np.float64)  # (IN, K)

    gmin = sg[:, 0].astype(np.float64)
    gmax = sg[:, -1].astype(np.float64)
    rng = np.maximum(gmax - gmin, 1e-6)
    gscale = (K - 1) / rng  # (IN,)
    gbias = -gmin * gscale

    # kink decomposition: f(w) = mc0 + sum_{j=0..11} D_j relu(w - j)
    s = mc[:, 1:] - mc[:, :-1]  # slopes, (IN, 11)
    D = np.empty((IN, NKINK), dtype=np.float64)
    D[:, 0] = s[:, 0]
    D[:, 1:11] = s[:, 1:] - s[:, :-1]
    D[:, 11] = -s[:, -1]
    D *= SPL_SCALE  # device computes SPL_SCALE*spline; wtb is descaled

    # normalized coordinate, feature-major; consumed by the spline chain AND
    # (rescaled weights) by the residual matmul
    w = (x.astype(np.float64) * gscale[None, :] + gbias[None, :]).T  # (IN, M)
    w16 = np.ascontiguousarray(w, dtype=F16)

    pwT = proj_w.astype(np.float64).T  # (IN, OUT)
    rwT = res_w.astype(np.float64).T  # (IN, OUT)

    def _tile_rows(a, dtype):
        return np.ascontiguousarray(
            a.reshape(FT, 128, OUT).transpose(1, 0, 2), dtype=dtype
        )

    wtb = _tile_rows(pwT / SPL_SCALE, FP8)
    wt8 = _tile_rows(rwT / gscale[:, None], F16)

    # bias fold: proj_b + mc0 @ pwT + gmin @ rwT (res path shift);
    # laid out [128, FT]: partition p of output-column block ob gets
    # bfold[128*ob + p] (consumed as a per-partition ACT bias at evac)
    bfold = proj_b.astype(np.float64) + mc[:, 0] @ pwT + gmin @ rwT
    bias = np.ascontiguousarray(
        bfold.reshape(FT, 128).T, dtype=np.float32
    )

    dcoef = np.ascontiguousarray(
        D.reshape(FT, 128, NKINK).transpose(1, 0, 2), dtype=np.float32
    )
    return w16, wtb, wt8, bias, dcoef


def _make_in_maps(inputs):
    w16, wtb, wt8, bias, dcoef = _prep(**inputs)
    in_maps = []
    for c in range(N_CORES):
        sl = slice(c * MC, (c + 1) * MC)
        in_maps.append(
            {
                "w16": np.ascontiguousarray(w16[:, sl]),
                "wtb": wtb,
                "wt8": wt8,
                "bias": bias,
                "dcoef": dcoef,
            }
        )
    return in_maps


def kernel(**inputs):
    from concourse.bass_utils import run_bass_kernel_spmd

    inputs = {k: np.asarray(v) for k, v in inputs.items()}
    nc = _build_graph()
    in_maps = _make_in_maps(inputs)
    res = run_bass_kernel_spmd(nc, in_maps, core_ids=list(range(N_CORES)))
    return np.concatenate(
        [res.results[c]["out"].T.astype(np.float32) for c in range(N_CORES)],
        axis=0,
    )


if __name__ == "__main__":
    rng = np.random.default_rng(0)
    fake = {
        "x": rng.standard_normal((M, IN), dtype=np.float32),
        "grid": rng.standard_normal((IN, K), dtype=np.float32),
        "coeffs": rng.standard_normal((IN, K), dtype=np.float32) * 0.1,
        "knot_alive": rng.standard_normal((IN, K), dtype=np.float32) + 3,
        "proj_w": rng.standard_normal((OUT, IN), dtype=np.float32) / 32,
        "proj_b": rng.standard_normal((OUT,), dtype=np.float32) * 0.01,
        "res_w": rng.standard_normal((IN, OUT), dtype=np.float32).T / 32,
    }
    y = kernel(**fake)
    print("kernel output", y.shape, y.dtype)


# revision 18
# speedup vs baseline: 1.1132x; 1.0516x over previous
"""AdaptiveSplineLayer on 8 Trainium2 NeuronCores (Bass/Tile).

Reference computation (per element, feature i, row m):
    sort grid[i], mc = (coeffs*sigmoid(alive)) sorted by grid order
    xn = clip((clip(x, gmin, gmax) - gmin) / range * 11, ...)
    spline = lerp of mc at floor(xn)
    out = spline @ proj_w.T + proj_b + x @ res_w.T

Kernel strategy (data-parallel over M; spline params + weights replicated):
  * Host: w = gscale*x + gbias (unclamped normalized coordinate, fp16,
    feature-major). The piecewise-linear spline with uniform knots is exactly
    f(w) = mc0 + sum_{j=0..11} D_j * relu(w - j); the two end kinks reproduce
    the clamping. mc0 folds into the output bias.
  * The SAME w tensor feeds the residual path: x = (w - gbias)/gscale, so
    x @ res_w.T = w @ (res_w/gscale).T + gmin @ res_w.T (constant -> bias).
    One 4MB fp16 input replaces the baseline's 8MB f32 + 4MB bf16 pair.
  * Device: 6 custom-DVE pair ops per feature tile evaluate all 12 kinks
    (2 kinks per 8-stage op), writing spline^T in fp8e4. TensorE contracts
    spline^T @ (proj_w/4)^T in fp8 DoubleRow and w^T @ (res_w/gscale)^T in
    fp16, PSUM-accumulated, seeded by a K=1 fp16 matmul carrying the folded
    bias. Output is written fp16 (4MB) and upcast to f32 on host.
"""

import os
import sys

import numpy as np

for _p in ("/opt/trn_rl_repo",):
    if _p not in sys.path and os.path.isdir(_p):
        sys.path.insert(0, _p)

import ml_dtypes

BF16 = ml_dtypes.bfloat16
FP8 = ml_dtypes.float8_e4m3fn
F16 = np.float16

M, IN, OUT, K = 16384, 1024, 1024, 12
N_CORES = 8
MC = M // N_CORES  # 2048 rows per core
FT = IN // 128  # 8 feature tiles
MT = MC // 128  # 16 m tiles per core
OC = OUT // 512  # 2 output column chunks
NKINK = K  # 12 kink terms j=0..11
SPL_SCALE = 4.0  # spline fp8 upscale (proj weights descaled to match)
ACT_KINKS = False  # kinks 4,5 via ACT |.| + Pool combine (DVE: 6 -> 5 passes)

# --------------------------------------------------------------------------
# Custom DVE ops: two relu-kinks per pass.
#   SPLINE_PAIR0   : out = s0*relu(in0-imm2) + s1*relu(in0-(imm2+1))
#   SPLINE_PAIR_ACC: out = in1 + s0*relu(in0-imm2) + s1*relu(in0-(imm2+1))
# --------------------------------------------------------------------------


def _register_spline_ops():
    from concourse.dve_ops import (
        CUSTOM_DVE_SPECS,
        OPS,
        _SUB_OPCODE_FOR_NAME,
        DveOp,
    )
    from concourse.dve_spec import (
        C0,
        C1,
        C2,
        One,
        Spec,
        Src0,
        Src1,
        _has_src1,
        lower,
        relu,
    )
    from concourse.dve_uop import DveOpSpec

    def _dve_relu(x):
        return np.maximum(
            np.nan_to_num(x, nan=0.0, posinf=np.inf, neginf=-np.inf), 0
        )

    def _ref_pair_acc(in0, in1, s0, s1, imm2):
        return (
            in1
            + s0 * _dve_relu(in0.astype(np.float32) - imm2)
            + s1 * _dve_relu(in0.astype(np.float32) - (imm2 + 1.0))
        ).astype(np.float32)

    def _ref_pair0(in0, in1, s0, s1, imm2):
        return (
            s0 * _dve_relu(in0.astype(np.float32) - imm2)
            + s1 * _dve_relu(in0.astype(np.float32) - (imm2 + 1.0))
        ).astype(np.float32)

    def _reg(name, spec):
        if name in _SUB_OPCODE_FOR_NAME:
            return next(o for o in OPS if o.name == name)
        row = max(_SUB_OPCODE_FOR_NAME.values()) + 1
        assert row < 0x20
        op = DveOp.__new__(DveOp)
        object.__setattr__(op, "name", name)
        object.__setattr__(op, "spec", spec)
        object.__setattr__(op, "subdim", False)
        object.__setattr__(op, "perf_en", {})
        s = DveOpSpec(
            name=name,
            opcode=row,
            uops=lower(spec, ver="v3"),
            rd1_en=_has_src1(spec),
        )
        object.__setattr__(op, "uops_sha", {"v3": s.sha("v3")})
        OPS.append(op)
        _SUB_OPCODE_FOR_NAME[name] = row
        CUSTOM_DVE_SPECS[name] = spec
        return op

    body_acc = (Src1 + C0 * relu(Src0 - C2)) + C1 * relu(Src0 - (C2 + One))
    body0 = C0 * relu(Src0 - C2) + C1 * relu(Src0 - (C2 + One))
    pair_acc = _reg("SPLINE_PAIR_ACC", Spec(body=body_acc, reference=_ref_pair_acc))
    pair0 = _reg("SPLINE_PAIR0", Spec(body=body0, reference=_ref_pair0))
    return pair0, pair_acc


# --------------------------------------------------------------------------
# Device graph
# --------------------------------------------------------------------------

_GRAPH_CACHE = {}


def _build_graph(m_split=(4, 4, 4, 4), repeat=1, acc_bufs=6, fori=False, skip=(),
                 wave_n=8, act_kinks=ACT_KINKS):
    if isinstance(m_split, (tuple, list)):
        m_split = tuple(m_split)
    key = (m_split, repeat, acc_bufs, fori, tuple(skip), wave_n, act_kinks)
    if key in _GRAPH_CACHE:
        return _GRAPH_CACHE[key]

    import concourse.bacc as bacc
    import concourse.mybir as mybir
    import concourse.tile as tile

    pair0, pair_acc = _register_spline_ops()

    dt = mybir.dt
    DR = mybir.MatmulPerfMode.DoubleRow
    Alu = mybir.AluOpType
    ABS_FN = mybir.ActivationFunctionType.Abs
    nc = bacc.Bacc("TRN2", target_bir_lowering=False, debug=False, num_devices=1)

    w16 = nc.dram_tensor("w16", [IN, MC], dt.float16, kind="ExternalInput")
    wtb = nc.dram_tensor("wtb", [128, FT, OUT], dt.float8e4, kind="ExternalInput")
    wt8 = nc.dram_tensor("wt8", [128, FT, OUT], dt.float16, kind="ExternalInput")
    bias = nc.dram_tensor("bias", [128, FT], dt.float32, kind="ExternalInput")
    dcoef = nc.dram_tensor("dcoef", [128, FT, NKINK], dt.float32, kind="ExternalInput")
    out = nc.dram_tensor("out", [OUT, MC], dt.float16, kind="ExternalOutput")

    with tile.TileContext(nc) as tc:
        with (
            tc.tile_pool(name="const", bufs=1) as const_pool,
            tc.tile_pool(name="wtp", bufs=1) as wt_pool,
            tc.tile_pool(name="persist", bufs=1) as persist_pool,
            tc.tile_pool(name="accp", bufs=acc_bufs) as acc_pool,
            tc.tile_pool(name="outp", bufs=4) as out_pool,
            tc.tile_pool(name="psum", bufs=8, space="PSUM") as psum_pool,
        ):
            # ---- small constants first (first DVE op needs dcoef) ----
            dc_t = const_pool.tile([128, FT, NKINK], dt.float32)
            nc.sync.dma_start(dc_t[:], dcoef[:])
            bias_t = const_pool.tile([128, FT], dt.float32)
            nc.sync.dma_start(bias_t[:], bias[:])
            mj_t = const_pool.tile([128, 2], dt.float32)
            nc.vector.memset(mj_t[:, 0:1], -4.0)
            nc.vector.memset(mj_t[:, 1:2], -5.0)

            wtb_t = wt_pool.tile([128, FT, OUT], dt.float8e4, tag="wtb")
            wt8_t = wt_pool.tile([128, FT, OUT], dt.float16, tag="wt8")

            # persistent matmul inputs: w16 doubles as DVE input + res operand
            w16_t = persist_pool.tile([128, FT, MC], dt.float16, tag="w16")
            spl_t = persist_pool.tile([128, FT, MC], dt.float8e4, tag="spl")

            from contextlib import nullcontext

            if "spline" in skip:
                nc.gpsimd.memset(spl_t[:], 0.0)
            if fori:
                # bench mode: weights are loop-invariant; load them up front
                nc.sync.dma_start(wtb_t[:], wtb[:])
                nc.sync.dma_start(wt8_t[:], wt8[:])
            rep_ctx = (
                tc.For_i(0, repeat, 1, staggered_reset=True)
                if fori
                else nullcontext()
            )
            blocks = (
                list(m_split)
                if isinstance(m_split, (tuple, list))
                else [MT // m_split] * m_split
            )
            assert sum(blocks) == MT
            starts = [sum(blocks[:i]) for i in range(len(blocks))]
            with rep_ctx:
              for _rep in range(1 if fori else repeat):
                for h, (b0, bn) in enumerate(zip(starts, blocks)):
                    sl = slice(b0 * 128, (b0 + bn) * 128)
                    MS = bn * 128
                    # chains in interleaved pairs: adjacent DVE ops belong to
                    # different chains, so the engine never stalls on its own
                    # predecessor's completion semaphore
                    tpairs = (
                        [(t, t + 1) for t in range(0, FT, 2)]
                        if "spline" not in skip
                        else []
                    )
                    if "spline" in skip:
                        # still stream w16 (res matmul input + DMA parity)
                        for t in range(FT):
                            nc.sync.dma_start(
                                w16_t[:, t, sl], w16[128 * t : 128 * (t + 1), sl]
                            )
                    for tp in tpairs:
                        dcss, accs = {}, {}
                        for t in tp:
                            nc.sync.dma_start(
                                w16_t[:, t, sl], w16[128 * t : 128 * (t + 1), sl]
                            )
                            dcss[t] = [
                                dc_t[:, t, j : j + 1] for j in range(NKINK)
                            ]
                        if act_kinks:
                            # kinks 4,5 via ACT Abs + Pool combine:
                            # D*relu(w-j) = (D/2)(w-j) + (D/2)|w-j|; linear
                            # part is folded into wt8/bias on the host, and
                            # dcoef slots 4,5 hold D~/2. P = sum of |.| terms
                            # enters the chain through pass 1's in1 slot.
                            ps4 = {}
                            for t in tp:
                                a4 = acc_pool.tile(
                                    [128, MS], dt.float16, tag="abs4"
                                )
                                nc.scalar.activation(
                                    a4[:], w16_t[:, t, sl], ABS_FN,
                                    bias=mj_t[:, 0:1], scale=1.0,
                                )
                                a5 = acc_pool.tile(
                                    [128, MS], dt.float16, tag="abs5"
                                )
                                nc.scalar.activation(
                                    a5[:], w16_t[:, t, sl], ABS_FN,
                                    bias=mj_t[:, 1:2], scale=1.0,
                                )
                                u4 = acc_pool.tile(
                                    [128, MS], dt.float16, tag="u4"
                                )
                                nc.vector.tensor_scalar(
                                    u4[:], a4[:], dcss[t][4], None,
                                    Alu.mult,
                                )
                                u5 = acc_pool.tile(
                                    [128, MS], dt.float16, tag="u5"
                                )
                                nc.vector.tensor_scalar(
                                    u5[:], a5[:], dcss[t][5], None,
                                    Alu.mult,
                                )
                                P = acc_pool.tile(
                                    [128, MS], dt.float16, tag="P"
                                )
                                nc.gpsimd.tensor_tensor(
                                    P[:], u4[:], u5[:], Alu.add
                                )
                                ps4[t] = P
                            qpairs = [0, 2, 6, 8, 10]
                            for t in tp:
                                accs[t] = ps4[t]
                        else:
                            qpairs = [0, 2, 4, 6, 8, 10]
                            for t in tp:
                                acc0 = acc_pool.tile(
                                    [128, MS], dt.float32, tag="acc"
                                )
                                accs[t] = acc0
                                nc.vector._custom_dve(
                                    pair0, out=accs[t][:], in0=w16_t[:, t, sl],
                                    s0=dcss[t][0], s1=dcss[t][1], imm2=0.0,
                                )
                            qpairs = qpairs[1:]
                        for j2 in qpairs[:-1]:
                            for t in tp:
                                nxt = acc_pool.tile(
                                    [128, MS], dt.float32, tag="acc"
                                )
                                nc.vector._custom_dve(
                                    pair_acc, out=nxt[:], in0=w16_t[:, t, sl],
                                    in1=accs[t][:], s0=dcss[t][j2],
                                    s1=dcss[t][j2 + 1], imm2=float(j2),
                                )
                                accs[t] = nxt
                        j2 = qpairs[-1]
                        for t in tp:
                            nc.vector._custom_dve(
                                pair_acc, out=spl_t[:, t, sl], in0=w16_t[:, t, sl],
                                in1=accs[t][:], s0=dcss[t][j2],
                                s1=dcss[t][j2 + 1], imm2=float(j2),
                            )
                    if h == 0 and not fori:
                        # weights: needed only once the first matmul group
                        # runs; keep them off the critical DMA path of the
                        # first spline block
                        nc.sync.dma_start(wtb_t[:], wtb[:])
                        nc.sync.dma_start(wt8_t[:], wt8[:])
                    # matmuls, weights-stationary: out^T[o, m] so each
                    # Ldweights (wt8/wtb column block) is reused across the
                    # block's m chunks; bias rides the ACT evacuation as a
                    # per-partition add (no bias matmul).
                    assert bn % 4 == 0, "blocks must align to 512-row chunks"
                    mchunks = [
                        slice(b0 * 128 + 512 * c, b0 * 128 + 512 * (c + 1))
                        for c in range(bn // 4)
                    ] if "mm" not in skip else []
                    ots = {}
                    for ob in range(FT):  # 8 output-column blocks of 128
                        opart = slice(128 * ob, 128 * (ob + 1))
                        if "mm" in skip:
                            break
                        ot = out_pool.tile(
                            [128, bn * 128], dt.float16, tag="evac"
                        )
                        ots[ob] = ot
                        pss = {}
                        for ci, msl in enumerate(mchunks):
                            ps = psum_pool.tile([128, 512], dt.float32)
                            pss[ci] = ps
                            # res path: one wt8 column-block load per t,
                            # streamed over this m chunk
                            for t in range(FT):
                                nc.tensor.matmul(
                                    ps[:], wt8_t[:, t, opart],
                                    w16_t[:, t, msl],
                                    start=(t == 0), stop=False,
                                )
                            # spline path fp8 DoubleRow
                            for q in range(FT // 2):
                                nc.tensor.matmul(
                                    ps[:],
                                    wtb_t[:, 2 * q : 2 * q + 2, opart],
                                    spl_t[:, 2 * q : 2 * q + 2, msl],
                                    start=False, stop=(q == FT // 2 - 1),
                                    perf_mode=DR,
                                )
                        # evacuate: ACT Identity adds per-partition bias and
                        # casts fp32->fp16 into the block's staging columns
                        for ci, msl in enumerate(mchunks):
                            nc.scalar.activation(
                                ots[ob][:, 512 * ci : 512 * (ci + 1)],
                                pss[ci][:],
                                mybir.ActivationFunctionType.Identity,
                                bias=bias_t[:, ob : ob + 1],
                            )
                        nc.gpsimd.dma_start(
                            out[opart, b0 * 128 : (b0 + bn) * 128], ots[ob][:]
                        )

            if "mm" in skip:
                with tc.tile_pool(name="sinkp", bufs=2) as sink_pool:
                    for ob in range(FT):
                        st = sink_pool.tile([128, MC], dt.float16, tag="sink")
                        nc.scalar.copy(st[:], spl_t[:, ob, :])
                        nc.sync.dma_start(out[128 * ob : 128 * (ob + 1), :], st[:])

    nc.compile()
    _GRAPH_CACHE[key] = nc
    return nc


# --------------------------------------------------------------------------
# Host-side parameter preparation
# --------------------------------------------------------------------------


def _prep(x, grid, coeffs, knot_alive, proj_w, proj_b, res_w):
    g64 = grid.astype(np.float64)
    order = np.argsort(g64, axis=1, kind="stable")
    sg = np.take_along_axis(grid.astype(np.float32), order, axis=1)
    # masked coeffs, sorted by grid order (sigmoid in f32 like the reference)
    mcu = coeffs.astype(np.float32) * (
        1.0 / (1.0 + np.exp(-knot_alive.astype(np.float32)))
    )
    mc = np.take_along_axis(mcu, order, axis=1).astype(np.float64)  # (IN, K)

    gmin = sg[:, 0].astype(np.float64)
    gmax = sg[:, -1].astype(np.float64)
    rng = np.maximum(gmax - gmin, 1e-6)
    gscale = (K - 1) / rng  # (IN,)
    gbias = -gmin * gscale

    # kink decomposition: f(w) = mc0 + sum_{j=0..11} D_j relu(w - j)
    s = mc[:, 1:] - mc[:, :-1]  # slopes, (IN, 11)
    D = np.empty((IN, NKINK), dtype=np.float64)
    D[:, 0] = s[:, 0]
    D[:, 1:11] = s[:, 1:] - s[:, :-1]
    D[:, 11] = -s[:, -1]

    # normalized coordinate, feature-major; consumed by the spline chain AND
    # (rescaled weights) by the residual matmul
    w = (x.astype(np.float64) * gscale[None, :] + gbias[None, :]).T  # (IN, M)
    w16 = np.ascontiguousarray(w, dtype=F16)

    pwT = proj_w.astype(np.float64).T  # (IN, OUT)
    rwT = res_w.astype(np.float64).T  # (IN, OUT)

    def _tile_rows(a, dtype):
        return np.ascontiguousarray(
            a.reshape(FT, 128, OUT).transpose(1, 0, 2), dtype=dtype
        )

    # bias fold: proj_b + mc0 @ pwT + gmin @ rwT (res path shift);
    # laid out [128, FT]: partition p of output-column block ob gets
    # bfold[128*ob + p] (consumed as a per-partition ACT bias at evac)
    bfold = proj_b.astype(np.float64) + mc[:, 0] @ pwT + gmin @ rwT
    vres = rwT / gscale[:, None]
    Dd = D * SPL_SCALE  # device-side spline is SPL_SCALE*spline
    if ACT_KINKS:
        # kinks 4,5 evaluated as (D/2)|w-j| on ACT; their linear halves
        # (D/2)(w-j) ride the res weights / bias instead
        vres = vres + ((D[:, 4] + D[:, 5]) / 2)[:, None] * pwT
        bfold = bfold + ((-4.0 * D[:, 4] - 5.0 * D[:, 5]) / 2) @ pwT
        Dd[:, 4] = 2.0 * D[:, 4]  # = (SPL_SCALE*D)/2
        Dd[:, 5] = 2.0 * D[:, 5]

    wtb = _tile_rows(pwT / SPL_SCALE, FP8)
    wt8 = _tile_rows(vres, F16)
    bias = np.ascontiguousarray(
        bfold.reshape(FT, 128).T, dtype=np.float32
    )

    dcoef = np.ascontiguousarray(
        Dd.reshape(FT, 128, NKINK).transpose(1, 0, 2), dtype=np.float32
    )
    return w16, wtb, wt8, bias, dcoef


def _make_in_maps(inputs):
    w16, wtb, wt8, bias, dcoef = _prep(**inputs)
    in_maps = []
    for c in range(N_CORES):
        sl = slice(c * MC, (c + 1) * MC)
        in_maps.append(
            {
                "w16": np.ascontiguousarray(w16[:, sl]),
                "wtb": wtb,
                "wt8": wt8,
                "bias": bias,
                "dcoef": dcoef,
            }
        )
    return in_maps


def kernel(**inputs):
    from concourse.bass_utils import run_bass_kernel_spmd

    inputs = {k: np.asarray(v) for k, v in inputs.items()}
    nc = _build_graph()
    in_maps = _make_in_maps(inputs)
    res = run_bass_kernel_spmd(nc, in_maps, core_ids=list(range(N_CORES)))
    return np.concatenate(
        [res.results[c]["out"].T.astype(np.float32) for c in range(N_CORES)],
        axis=0,
    )


if __name__ == "__main__":
    rng = np.random.default_rng(0)
    fake = {
        "x": rng.standard_normal((M, IN), dtype=np.float32),
        "grid": rng.standard_normal((IN, K), dtype=np.float32),
        "coeffs": rng.standard_normal((IN, K), dtype=np.float32) * 0.1,
        "knot_alive": rng.standard_normal((IN, K), dtype=np.float32) + 3,
        "proj_w": rng.standard_normal((OUT, IN), dtype=np.float32) / 32,
        "proj_b": rng.standard_normal((OUT,), dtype=np.float32) * 0.01,
        "res_w": rng.standard_normal((IN, OUT), dtype=np.float32).T / 32,
    }
    y = kernel(**fake)
    print("kernel output", y.shape, y.dtype)
